# revision 1
# baseline (speedup 1.0000x reference)
"""Trainium2 Bass kernel for GCNN operator:
    h   = einsum('bnf,nfg->bng', x, kernel)   # per-node feature transform
    out = einsum('nm,bmg->bng', A, h) + bias  # dense adjacency aggregation

Sharding: node dim N row-sharded across 8 cores. Each core computes h for
its 2048 nodes (DVE), AllGathers h (small), then computes its row-shard of
A @ H on the TensorEngine while streaming its A-shard (pre-transposed and
cast to fp16 on host) from HBM at full DMA bandwidth.

Self-contained: hardcodes shapes; only imports concourse + numpy/jax.
"""

import numpy as np

B, N, F, G = 2, 16384, 16, 16
NCORES = 8
P = 128                    # SBUF partitions
C = B * G                  # 32 fused (batch, out-feature) columns
NT = 512                   # matmul moving-operand free-dim per instruction


def build_nc(n=N, ncores=NCORES, at_bufs=6, km=4):
    """Build the per-core Bass program (SPMD: same program on all cores)."""
    import concourse.bass as bass
    import concourse.mybir as mybir
    import concourse.tile as tile
    from concourse import bacc
    from concourse.masks import make_identity

    f32 = mybir.dt.float32
    f16 = mybir.dt.float16

    nl = n // ncores           # nodes per core
    j_n = nl // P              # local node blocks (16 at full size)
    mj = n // P                # contraction blocks (128 at full size)
    nt_n = max(nl // NT, 1)    # rhs free-dim chunks per at-tile
    ntc = min(NT, nl)
    km = min(km, mj)           # contraction blocks batched per DMA

    nc = bacc.Bacc(
        "TRN2", target_bir_lowering=False, debug=False, num_devices=ncores
    )

    at = nc.dram_tensor("at", [n, nl], f16, kind="ExternalInput")
    xs = nc.dram_tensor("xs", [B, nl, F], f16, kind="ExternalInput")
    ks = nc.dram_tensor("ks", [nl, F, G], f16, kind="ExternalInput")
    bs = nc.dram_tensor("bs", [nl, G], f32, kind="ExternalInput")
    outs = nc.dram_tensor("outs", [P, j_n * C], f32, kind="ExternalOutput")

    with tile.TileContext(nc) as tc:
        with (
            tc.tile_pool(name="dram", bufs=1, space="DRAM") as dram,
            tc.tile_pool(name="const", bufs=1) as const,
            tc.tile_pool(name="work", bufs=2) as work,
            tc.tile_pool(name="atp", bufs=at_bufs) as atp,
            tc.tile_pool(name="pacc", bufs=1, space="PSUM") as pacc,
            tc.tile_pool(name="ptr", bufs=2, space="PSUM") as ptr,
        ):
            # h bounce/gather buffers are partition-major: [P, j_n*C] per rank,
            # AllGather concats ranks on axis 0 -> [ncores*P, j_n*C]. Global
            # m-block r*j_n + j lands at rows [r*P:(r+1)*P], cols [j*C:(j+1)*C],
            # i.e. hq free-dim order (r, j, c) == m-block-major, matching the
            # matmul's hq[:, m*C:(m+1)*C] slicing.
            w_h = j_n * C
            h_bounce = dram.tile([P, w_h], f16)
            h_full = dram.tile([ncores * P, w_h], f16, addr_space="Shared")

            # ---- prologue loads (SWDGE queue, separate from the A stream) ----
            x_sb = const.tile([P, j_n, B, F], f16)
            for b in range(B):
                nc.gpsimd.dma_start(
                    out=x_sb[:, :, b, :],
                    in_=xs[b].rearrange("(j p) f -> p j f", p=P),
                )
            k_sb = const.tile([P, j_n, F, G], f16)
            nc.gpsimd.dma_start(
                out=k_sb[:, :, :, :],
                in_=ks.ap().rearrange("(j p) f g -> p j f g", p=P),
            )
            bias2 = const.tile([P, j_n, C], f32)
            for b in range(B):
                nc.gpsimd.dma_start(
                    out=bias2[:, :, b * G : (b + 1) * G],
                    in_=bs.ap().rearrange("(j p) g -> p j g", p=P),
                )
            identity = const.tile([C, C], f32)
            make_identity(nc, identity[:, :])

            # ---- h = einsum('bnf,nfg->bng') on DVE, local nodes ----
            h_f32 = const.tile([P, j_n, B, G], f32)
            for b in range(B):
                prod = work.tile([P, j_n, G, F], f32, tag="prod")
                nc.vector.tensor_tensor(
                    prod[:, :, :, :],
                    x_sb[:, :, b, None, :].to_broadcast([P, j_n, G, F]),
                    k_sb[:, :, :, :].rearrange("p j f g -> p j g f"),
                    mybir.AluOpType.mult,
                )
                nc.vector.tensor_reduce(
                    h_f32[:, :, b, :],
                    prod[:, :, :, :],
                    axis=mybir.AxisListType.X,
                    op=mybir.AluOpType.add,
                )
            h16 = const.tile([P, j_n, B, G], f16)
            nc.vector.tensor_copy(h16[:, :, :, :], h_f32[:, :, :, :])
            nc.gpsimd.dma_start(
                out=h_bounce[:, :],
                in_=h16[:, :, :, :].rearrange("p j b g -> p (j b g)"),
            )

            # ---- AllGather h -> full H [n, C] on every core ----
            nc.gpsimd.collective_compute(
                "AllGather",
                mybir.AluOpType.bypass,
                replica_groups=[list(range(ncores))],
                ins=[h_bounce[:, :].opt()],
                outs=[h_full[:, :].opt()],
            )
            hq = const.tile([P, mj * C], f16)
            nc.gpsimd.dma_start(
                out=hq[:, :].rearrange("p (r w) -> p r w", w=w_h),
                in_=h_full.rearrange("(r p) w -> p r w", p=P),
            )

            # ---- main loop: out^T[c, n_local] += H_m^T-block @ A^T tile ----
            acc = [
                pacc.tile([C, ntc], f32, tag=f"acc{t}", name=f"acc{t}")
                for t in range(nt_n)
            ]
            # A^T stream: km contraction blocks per DMA (amortizes per-DMA
            # fixed cost), alternating between the two HWDGE rings (SP/ACT).
            at_r = at.ap().rearrange("(mb km p) nl -> mb p km nl", p=P, km=km)
            for mb in range(mj // km):
                eng = nc.scalar if mb % 2 else nc.sync
                at_t = atp.tile([P, km, nl], f16, tag="at_t", name="at_t")
                eng.dma_start(out=at_t[:, :, :], in_=at_r[mb])
                for kk in range(km):
                    m = mb * km + kk
                    for t in range(nt_n):
                        nc.tensor.matmul(
                            acc[t][:, :],
                            hq[:, m * C : (m + 1) * C],
                            at_t[:, kk, t * ntc : (t + 1) * ntc],
                            start=(m == 0),
                            stop=(m == mj - 1),
                        )

            # ---- epilogue: transpose out^T back to [node, c], add bias ----
            outT = work.tile([C, nl], f32, tag="outT")
            for t in range(nt_n):
                nc.vector.tensor_copy(outT[:, t * ntc : (t + 1) * ntc], acc[t][:, :])
            out_sb = work.tile([P, j_n, C], f32, tag="out_sb")
            for j in range(j_n):
                pt = ptr.tile([P, C], f32, tag="pt", name="pt")
                nc.tensor.transpose(
                    pt[:, :], outT[:, j * P : (j + 1) * P], identity[:, :]
                )
                nc.vector.tensor_add(out_sb[:, j, :], pt[:, :], bias2[:, j, :])
            nc.sync.dma_start(
                out=outs.ap(), in_=out_sb[:, :, :].rearrange("p j c -> p (j c)")
            )

    nc.compile()
    return nc


_NC_CACHE = {}


def _get_nc(n=N, ncores=NCORES):
    key = (n, ncores)
    if key not in _NC_CACHE:
        _NC_CACHE[key] = build_nc(n, ncores)
    return _NC_CACHE[key]


def make_in_maps(x, A, kern, bias, n=N, ncores=NCORES):
    nl = n // ncores
    in_maps = []
    for r in range(ncores):
        sl = slice(r * nl, (r + 1) * nl)
        in_maps.append(
            {
                "at": A[sl, :].T.astype(np.float16),
                "xs": np.ascontiguousarray(x[:, sl, :]).astype(np.float16),
                "ks": np.ascontiguousarray(kern[sl]).astype(np.float16),
                "bs": np.ascontiguousarray(bias[sl]),
            }
        )
    return in_maps


def assemble_out(results, n=N, ncores=NCORES):
    nl = n // ncores
    j_n = nl // P
    parts = []
    for r in range(ncores):
        o = results[r]["outs"].reshape(P, j_n, B, G)
        parts.append(o.transpose(2, 1, 0, 3).reshape(B, nl, G))
    return np.ascontiguousarray(np.concatenate(parts, axis=1))


def run(inputs, n=N, ncores=NCORES, trace=False, **spmd_kwargs):
    from concourse.bass_utils import run_bass_kernel_spmd

    x = np.asarray(inputs["x"], dtype=np.float32)
    A = np.asarray(inputs["A"], dtype=np.float32)
    kern = np.asarray(inputs["kernel"], dtype=np.float32)
    bias = np.asarray(inputs["bias"], dtype=np.float32)
    nc = _get_nc(n, ncores)
    in_maps = make_in_maps(x, A, kern, bias, n, ncores)
    res = run_bass_kernel_spmd(
        nc, in_maps, list(range(ncores)), trace=trace, **spmd_kwargs
    )
    out = assemble_out(res.results, n, ncores)
    return out, res


def kernel(**inputs) -> np.ndarray:
    out, _ = run(inputs)
    return out



# revision 8
# speedup vs baseline: 1.5247x; 1.5247x over previous
"""Trainium2 Bass kernel for GCNN operator:
    h   = einsum('bnf,nfg->bng', x, kernel)   # per-node feature transform
    out = einsum('nm,bmg->bng', A, h) + bias  # dense adjacency aggregation

Sharding: node dim N row-sharded across 8 cores. Each core computes h for
its 2048 nodes (DVE, fp16), AllGathers h (1MB), then computes its row-shard
of A @ H on the TensorEngine while streaming its A-shard from HBM.

v2: A^T streamed as fp8 E3M4 (halves HBM bytes; rel-err ~1.2e-2 vs the 2e-2
gate), 4-way column-tiled matmuls (fp16 stationary hq x fp8 moving A) into a
single PSUM bank, host-side bias/out layout so the epilogue is one DVE add,
and a dummy 1-byte collective issued at t=0 to absorb the one-time ncfw
barrier that otherwise serializes ~42us in front of the h AllGather.

Self-contained: hardcodes shapes; only imports concourse + numpy.
"""

import numpy as np

B, N, F, G = 2, 16384, 16, 16
NCORES = 8
P = 128                    # SBUF partitions
C = B * G                  # 32 fused (batch, out-feature) columns


def build_nc(n=N, ncores=NCORES, at_bufs=7, km=8):
    """Build the per-core Bass program (SPMD: same program on all cores)."""
    import concourse.bass as bass
    import concourse.mybir as mybir
    import concourse.tile as tile
    from concourse import bacc

    f32 = mybir.dt.float32
    f16 = mybir.dt.float16
    f8 = mybir.dt.float8e3
    u8 = mybir.dt.uint8

    nl = n // ncores           # nodes per core
    j_n = nl // P              # local node blocks (16 at full size)
    mj = n // P                # contraction blocks (128 at full size)
    ntc = max(nl // 4, 1)      # nodes per PSUM col-group (512 at full size)
    km = min(km, mj)           # contraction blocks batched per DMA

    nc = bacc.Bacc(
        "TRN2", target_bir_lowering=False, debug=False, num_devices=ncores
    )

    at = nc.dram_tensor("at", [mj // km, P, km, nl], f8, kind="ExternalInput")
    xs = nc.dram_tensor("xs", [B, nl, F], f16, kind="ExternalInput")
    ks = nc.dram_tensor("ks", [nl, F, G], f16, kind="ExternalInput")
    bs = nc.dram_tensor("bs", [P, ntc], f32, kind="ExternalInput")
    outs = nc.dram_tensor("outs", [P, ntc], f32, kind="ExternalOutput")

    with tile.TileContext(nc) as tc:
        with (
            tc.tile_pool(name="dram", bufs=1, space="DRAM") as dram,
            tc.tile_pool(name="const", bufs=1) as const,
            tc.tile_pool(name="work", bufs=2) as work,
            tc.tile_pool(name="atp", bufs=at_bufs) as atp,
            tc.tile_pool(name="pacc", bufs=1, space="PSUM") as pacc,
        ):
            # Dummy collective with no input deps: triggers at t~0 and absorbs
            # the one-time ncfw init barrier so the real AllGather doesn't pay
            # it on the critical path.
            dummy_in = dram.tile([1, 1], u8)
            dummy_out = dram.tile([ncores, 1], u8, addr_space="Shared")
            nc.gpsimd.collective_compute(
                "AllGather",
                mybir.AluOpType.bypass,
                replica_groups=[list(range(ncores))],
                ins=[dummy_in[:, :].opt()],
                outs=[dummy_out[:, :].opt()],
            )

            # h bounce/gather buffers are partition-major: [P, j_n*C] per rank,
            # AllGather concats ranks on axis 0 -> [ncores*P, j_n*C]. Global
            # m-block r*j_n + j lands at rows [r*P:(r+1)*P], cols [j*C:(j+1)*C],
            # i.e. hq free-dim order (r, j, c) == m-block-major, matching the
            # matmul's hq[:, m*C:(m+1)*C] slicing.
            w_h = j_n * C
            h_bounce = dram.tile([P, w_h], f16)
            h_full = dram.tile([ncores * P, w_h], f16, addr_space="Shared")

            # ---- prologue loads (SWDGE queue, separate from the A stream) ----
            x_sb = const.tile([P, j_n, B, F], f16)
            for b in range(B):
                nc.gpsimd.dma_start(
                    out=x_sb[:, :, b, :],
                    in_=xs[b].rearrange("(j p) f -> p j f", p=P),
                )
            k_sb = const.tile([P, j_n, F, G], f16)
            nc.gpsimd.dma_start(
                out=k_sb[:, :, :, :],
                in_=ks.ap().rearrange("(j p) f g -> p j f g", p=P),
            )
            bias_sb = const.tile([P, ntc], f32)
            nc.gpsimd.dma_start(out=bias_sb[:, :], in_=bs.ap())

            # ---- h = einsum('bnf,nfg->bng') on DVE (fp16), local nodes ----
            # fp16 accumulation over F=16 terms of ~0.02 magnitude is well
            # within the e3m4 A-stream error budget (~1.2e-2 rel).
            h16 = const.tile([P, j_n, B, G], f16)
            with nc.allow_low_precision(reason="16-term fp16 sum, err << fp8 A"):
                for b in range(B):
                    prod = work.tile([P, j_n, G, F], f16, tag="prod")
                    nc.vector.tensor_tensor(
                        prod[:, :, :, :],
                        x_sb[:, :, b, None, :].to_broadcast([P, j_n, G, F]),
                        k_sb[:, :, :, :].rearrange("p j f g -> p j g f"),
                        mybir.AluOpType.mult,
                    )
                    nc.vector.tensor_reduce(
                        h16[:, :, b, :],
                        prod[:, :, :, :],
                        axis=mybir.AxisListType.X,
                        op=mybir.AluOpType.add,
                    )
            nc.gpsimd.dma_start(
                out=h_bounce[:, :],
                in_=h16[:, :, :, :].rearrange("p j b g -> p (j b g)"),
            )

            # ---- AllGather h -> full H [n, C] on every core ----
            nc.gpsimd.collective_compute(
                "AllGather",
                mybir.AluOpType.bypass,
                replica_groups=[list(range(ncores))],
                ins=[h_bounce[:, :].opt()],
                outs=[h_full[:, :].opt()],
            )
            hq = const.tile([P, mj * C], f16)
            nc.gpsimd.dma_start(
                out=hq[:, :].rearrange("p (r w) -> p r w", w=w_h),
                in_=h_full.rearrange("(r p) w -> p r w", p=P),
            )

            # ---- main loop: 4-way col-tiled out^T accumulation ----
            # One PSUM bank [P, ntc] f32: partition group 32t+c holds
            # out^T[c, t*ntc + i] (c = b*G+g fused column, i free index).
            acc = pacc.tile([P, ntc], f32, tag="acc", name="acc")
            at_stream = at.ap()
            for mb in range(mj // km):
                eng = nc.scalar if mb % 2 else nc.sync
                at_t = atp.tile([P, km, nl], f8, tag="at_t", name="at_t")
                eng.dma_start(out=at_t[:, :, :], in_=at_stream[mb])
                for kk in range(km):
                    m = mb * km + kk
                    for t in range(4):
                        nc.tensor.matmul(
                            acc[32 * t : 32 * (t + 1), :],
                            hq[:, m * C : (m + 1) * C],
                            at_t[:, kk, t * ntc : (t + 1) * ntc],
                            start=(m == 0),
                            stop=(m == mj - 1),
                            tile_position=(0, 32 * t),
                        )

            # ---- epilogue: out = acc + bias (both already in PSUM layout) ----
            out_sb = work.tile([P, ntc], f32, tag="out_sb")
            nc.vector.tensor_tensor(
                out_sb[:, :], acc[:, :], bias_sb[:, :], mybir.AluOpType.add
            )
            nc.sync.dma_start(out=outs.ap(), in_=out_sb[:, :])

    nc.compile()
    return nc


_NC_CACHE = {}


def _get_nc(n=N, ncores=NCORES):
    key = (n, ncores)
    if key not in _NC_CACHE:
        _NC_CACHE[key] = build_nc(n, ncores)
    return _NC_CACHE[key]


def make_in_maps(x, A, kern, bias, n=N, ncores=NCORES, km=8):
    import ml_dtypes

    f8 = ml_dtypes.float8_e3m4
    nl = n // ncores
    mj = n // P
    km = min(km, mj)
    ntc = max(nl // 4, 1)
    in_maps = []
    for r in range(ncores):
        sl = slice(r * nl, (r + 1) * nl)
        # at[mb, p, kk, j] = A[sl.start + j, (mb*km + kk)*P + p], fp8 e3m4
        at = np.ascontiguousarray(A[sl, :].T)  # [n, nl]
        at = at.reshape(mj // km, km, P, nl).transpose(0, 2, 1, 3)
        at = np.ascontiguousarray(at).astype(f8)
        # bias in PSUM layout: bs[32t + b*G + g, i] = bias[sl.start + t*ntc + i, g]
        bl = bias[sl].reshape(4, ntc, G).transpose(0, 2, 1)  # [t, g, i]
        bs = np.ascontiguousarray(
            np.broadcast_to(bl[:, None], (4, B, G, ntc)).reshape(P, ntc)
        ).astype(np.float32)
        in_maps.append(
            {
                "at": at,
                "xs": np.ascontiguousarray(x[:, sl, :]).astype(np.float16),
                "ks": np.ascontiguousarray(kern[sl]).astype(np.float16),
                "bs": bs,
            }
        )
    return in_maps


def assemble_out(results, n=N, ncores=NCORES):
    nl = n // ncores
    ntc = max(nl // 4, 1)
    parts = []
    for r in range(ncores):
        o = results[r]["outs"].reshape(4, B, G, ntc)
        # out[b, t*ntc + i, g] = o[t, b, g, i]
        blk = np.ascontiguousarray(o.transpose(1, 0, 3, 2)).reshape(B, nl, G)
        parts.append(blk)
    return np.ascontiguousarray(np.concatenate(parts, axis=1))


def run(inputs, n=N, ncores=NCORES, trace=False, **spmd_kwargs):
    from concourse.bass_utils import run_bass_kernel_spmd

    x = np.asarray(inputs["x"], dtype=np.float32)
    A = np.asarray(inputs["A"], dtype=np.float32)
    kern = np.asarray(inputs["kernel"], dtype=np.float32)
    bias = np.asarray(inputs["bias"], dtype=np.float32)
    nc = _get_nc(n, ncores)
    in_maps = make_in_maps(x, A, kern, bias, n, ncores)
    res = run_bass_kernel_spmd(
        nc, in_maps, list(range(ncores)), trace=trace, **spmd_kwargs
    )
    out = assemble_out(res.results, n, ncores)
    return out, res


def kernel(**inputs) -> np.ndarray:
    out, _ = run(inputs)
    return out
